# revision 21
# baseline (speedup 1.0000x reference)
"""RNN-T joint network kernel for 8 Trainium2 NeuronCores.

logits[b,t,u,v] = tanh(enc_out[b,t]@W_enc + b_enc + pred_out[b,u]@W_pred + b_pred) @ W_joint + b_joint

Sharding: T axis split 8 ways (32 t's per core). Each core computes its
(B=4, T/8=32, U=64, V=1024) logit slab independently; no collectives.

Per-core dataflow (activations kept transposed, J on partitions, bf16
compute with f32 PSUM accumulate and f32 output):
  encT/predT   : PE transpose of the bf16 input slabs
  enc/pred proj: PE bf16 matmuls (W natural layout as lhsT)
  joint        : DVE broadcast-add (0-step APs) + ACT tanh -> bf16 slab
  logits       : PE bf16 matmuls, w_joint streaming as moving operand,
                 out tile [128 btu, 1024 v]; DVE fused PSUM-evac + b_joint
  store        : HWDGE DMA, 512KB contiguous per tile
"""

import sys

for _p in ("/opt/trn_rl_repo", "/root/.axon_site/_ro/trn_rl_repo"):
    if _p not in sys.path:
        sys.path.insert(0, _p)

import numpy as np

import concourse.tile as tile
from concourse import bacc, mybir
from concourse import bass_utils
from concourse.ap import AP
from concourse.masks import make_identity

F32 = mybir.dt.float32
BF16 = mybir.dt.bfloat16
NP_BF16 = mybir.dt.np(BF16)
TANH = mybir.ActivationFunctionType.Tanh
ADD = mybir.AluOpType.add

N_CORES = 8
B, T, U = 4, 256, 64
TL = T // N_CORES          # 32 t's per core
E = P = J = 512
V = 1024
JC = J // 128              # 4 j-chunks
BT = B * TL                # 128 (b,t) rows per core
NT = BT * U // 128         # 64 output tiles of 128 btu rows
TG = 4                     # t-groups per b (8 t's each -> slab of 512 btu)

_cache = {}


def _build():
    nc = bacc.Bacc("TRN2", target_bir_lowering=False, debug=False,
                   num_devices=N_CORES)

    enc_d = nc.dram_tensor("enc", [BT, E], BF16, kind="ExternalInput").ap()
    pred_d = nc.dram_tensor("pred", [B * U, P], BF16, kind="ExternalInput").ap()
    w_enc_d = nc.dram_tensor("w_enc", [E, J], BF16, kind="ExternalInput").ap()
    w_pred_d = nc.dram_tensor("w_pred", [P, J], BF16, kind="ExternalInput").ap()
    w_joint_d = nc.dram_tensor("w_joint", [J, V], BF16, kind="ExternalInput").ap()
    bias_ep_d = nc.dram_tensor("bias_ep", [128, JC], F32, kind="ExternalInput").ap()
    bias_j_d = nc.dram_tensor("bias_j", [128, V], F32, kind="ExternalInput").ap()
    out_d = nc.dram_tensor("out", [BT * U, V], F32, kind="ExternalOutput").ap()

    with tile.TileContext(nc) as tc:
        with (
            tc.tile_pool(name="const", bufs=1) as cp,
            tc.tile_pool(name="psum_sm", bufs=2, space="PSUM") as ps_sm,
            tc.tile_pool(name="psum_mm", bufs=6, space="PSUM") as ps_mm,
            tc.tile_pool(name="slab", bufs=4) as slab_pool,
            tc.tile_pool(name="outp", bufs=6) as out_pool,
        ):
            # ---- constant loads, ordered by when the pipeline needs them ----
            enc_sb = cp.tile([128, E], BF16)
            nc.sync.dma_start(enc_sb[:], enc_d[:])
            pred_sb = cp.tile([128, 2, P], BF16)
            nc.sync.dma_start(
                pred_sb[:], pred_d.rearrange("(r p) e -> p r e", p=128))
            w_enc_sb = cp.tile([128, JC, J], BF16)
            we_re = w_enc_d.rearrange("(c p) j -> p c j", p=128)
            for c in range(JC):
                nc.sync.dma_start(w_enc_sb[:, c, :], we_re[:, c, :])
            w_pred_sb = cp.tile([128, JC, J], BF16)
            wp_re = w_pred_d.rearrange("(c p) j -> p c j", p=128)
            for c in range(JC):
                nc.sync.dma_start(w_pred_sb[:, c, :], wp_re[:, c, :])
            bias_ep_sb = cp.tile([128, JC], F32)
            nc.sync.dma_start(bias_ep_sb[:], bias_ep_d[:])
            w_joint_sb = cp.tile([128, JC, V], BF16)
            wj_re = w_joint_d.rearrange("(c p) v -> p c v", p=128)
            for vh in range(2):
                nc.sync.dma_start(w_joint_sb[:, :, vh * 512:(vh + 1) * 512],
                                  wj_re[:, :, vh * 512:(vh + 1) * 512])
            bias_j_sb = cp.tile([128, V], F32)
            nc.sync.dma_start(bias_j_sb[:], bias_j_d[:])

            ident = cp.tile([128, 128], BF16)
            make_identity(nc, ident[:])

            # ---- transpose enc/pred to [E-part, row] ----
            encT = cp.tile([128, JC, BT], BF16)
            for c in range(JC):
                tr = ps_sm.tile([128, 512], BF16, tag="sm")
                nc.tensor.transpose(tr[:, 0:128], enc_sb[:, c * 128:(c + 1) * 128],
                                    ident[:])
                nc.scalar.copy(encT[:, c, :], tr[:, 0:128])
            predT = cp.tile([128, JC, B * U], BF16)
            for r in range(2):
                for c in range(JC):
                    tr = ps_sm.tile([128, 512], BF16, tag="sm")
                    nc.tensor.transpose(tr[:, 0:128],
                                        pred_sb[:, r, c * 128:(c + 1) * 128],
                                        ident[:])
                    nc.scalar.copy(predT[:, c, r * 128:(r + 1) * 128], tr[:, 0:128])

            # ---- projections (transposed layout [j, row]) ----
            encP = cp.tile([128, JC, BT], BF16)
            for jc in range(JC):
                pe = ps_mm.tile([128, 512], F32, tag="mm")
                for ec in range(JC):
                    nc.tensor.matmul(pe[:, 0:BT],
                                     w_enc_sb[:, ec, jc * 128:(jc + 1) * 128],
                                     encT[:, ec, :],
                                     start=(ec == 0), stop=(ec == JC - 1))
                nc.scalar.copy(encP[:, jc, :], pe[:, 0:BT])
            predP = cp.tile([128, JC, B * U], BF16)
            for jc in range(JC):
                pp = ps_mm.tile([128, 512], F32, tag="mm")
                for ec in range(JC):
                    nc.tensor.matmul(pp[:, 0:256],
                                     w_pred_sb[:, ec, jc * 128:(jc + 1) * 128],
                                     predT[:, ec, :],
                                     start=(ec == 0), stop=(ec == JC - 1))
                # fused (b_enc + b_pred) bias add during PSUM evacuation
                nc.vector.tensor_scalar_add(predP[:, jc, :], pp[:, 0:256],
                                            bias_ep_sb[:, jc:jc + 1])

            # ---- main loop: per (b, t-group of 8) slab -> 4 out tiles ----
            for b in range(B):
                for tg in range(TG):
                    bt0 = b * TL + tg * 8
                    slab = slab_pool.tile([128, JC, 512], BF16)
                    for jc in range(JC):
                        # [128, 8t, 64u] = pred[., u] (bcast t) + enc[., t] (bcast u)
                        p_ap = predP[:, jc, b * U:(b + 1) * U]
                        in0 = AP(p_ap.tensor, p_ap.offset,
                                 [p_ap.ap[0], [0, 8], [1, U]])
                        e_ap = encP[:, jc, bt0:bt0 + 8]
                        in1 = AP(e_ap.tensor, e_ap.offset,
                                 [e_ap.ap[0], [1, 8], [0, U]])
                        dst = slab[:, jc, :].rearrange("p (t u) -> p t u", t=8)
                        nc.vector.tensor_tensor(dst, in0, in1, ADD)
                        nc.scalar.activation(slab[:, jc, :], slab[:, jc, :], TANH)

                    for pi in range(4):
                        tidx = (b * TG + tg) * 4 + pi
                        ot = out_pool.tile([128, V], F32)
                        for vh in range(2):
                            vs = slice(vh * 512, (vh + 1) * 512)
                            po = ps_mm.tile([128, 512], F32, tag="mm")
                            for jc in range(JC):
                                nc.tensor.matmul(
                                    po[:],
                                    slab[:, jc, pi * 128:(pi + 1) * 128],
                                    w_joint_sb[:, jc, vs],
                                    start=(jc == 0), stop=(jc == JC - 1))
                            if tidx % 2 == 0 or tidx >= NT - 4:
                                # DVE: fused PSUM evac + b_joint add
                                nc.vector.tensor_tensor(
                                    ot[:, vs], po[:], bias_j_sb[:, vs], ADD)
                            else:
                                # ACT evacuates, GPSIMD adds bias in place
                                nc.scalar.copy(ot[:, vs], po[:])
                                nc.gpsimd.tensor_add(
                                    ot[:, vs], ot[:, vs], bias_j_sb[:, vs])
                        nc.sync.dma_start(
                            out_d[tidx * 128:(tidx + 1) * 128, :], ot[:])
    nc.compile()
    return nc


def _get_nc():
    if "nc" not in _cache:
        _cache["nc"] = _build()
    return _cache["nc"]


def make_in_maps(enc_out, pred_out, W_enc, b_enc, W_pred, b_pred, W_joint, b_joint):
    pred = np.ascontiguousarray(
        np.asarray(pred_out, dtype=np.float32).reshape(B * U, P)).astype(NP_BF16)
    bias_ep = np.ascontiguousarray(
        (np.asarray(b_enc, dtype=np.float32)
         + np.asarray(b_pred, dtype=np.float32)).reshape(JC, 128).T)
    bias_j = np.ascontiguousarray(
        np.broadcast_to(np.asarray(b_joint, dtype=np.float32), (128, V)))
    w_enc = np.ascontiguousarray(W_enc, dtype=np.float32).astype(NP_BF16)
    w_pred = np.ascontiguousarray(W_pred, dtype=np.float32).astype(NP_BF16)
    w_joint = np.ascontiguousarray(W_joint, dtype=np.float32).astype(NP_BF16)
    enc_f32 = np.asarray(enc_out, dtype=np.float32)
    in_maps = []
    for i in range(N_CORES):
        enc_slab = np.ascontiguousarray(
            enc_f32[:, i * TL:(i + 1) * TL, :].reshape(BT, E)).astype(NP_BF16)
        in_maps.append({
            "enc": enc_slab, "pred": pred,
            "w_enc": w_enc, "w_pred": w_pred, "w_joint": w_joint,
            "bias_ep": bias_ep, "bias_j": bias_j,
        })
    return in_maps


def assemble(results):
    return np.concatenate(
        [r["out"].reshape(B, TL, U, V) for r in results], axis=1)


def _axon_active():
    try:
        from concourse.bass_utils import axon_active
        return axon_active()
    except Exception:
        return False


def _get_fast_runner(nc):
    """Cached jit dispatch (axon path). Same mechanism as
    bass2jax.run_bass_via_pjrt, built once so repeat kernel() calls skip
    the per-call trace/lower/compile."""
    if "runner" in _cache:
        return _cache["runner"]

    import jax
    from jax.sharding import Mesh, PartitionSpec, NamedSharding
    from jax.experimental.shard_map import shard_map
    from concourse.bass2jax import (
        _bass_exec_p, install_neuronx_cc_hook, partition_id_tensor)

    install_neuronx_cc_hook()
    partition_name = nc.partition_id_tensor.name if nc.partition_id_tensor else None
    in_names, out_names, out_avals, zero_outs = [], [], [], []
    for alloc in nc.m.functions[0].allocations:
        if not isinstance(alloc, mybir.MemoryLocationSet):
            continue
        name = alloc.memorylocations[0].name
        if alloc.kind == "ExternalInput":
            if name != partition_name:
                in_names.append(name)
        elif alloc.kind == "ExternalOutput":
            shape = tuple(alloc.tensor_shape)
            dtype = mybir.dt.np(alloc.dtype)
            out_names.append(name)
            out_avals.append(jax.core.ShapedArray(shape, dtype))
            zero_outs.append(np.zeros(shape, dtype))
    n_params = len(in_names)
    n_outs = len(out_avals)
    all_names = in_names + out_names
    if partition_name is not None:
        all_names = all_names + [partition_name]

    def _body(*args):
        operands = list(args)
        if partition_name is not None:
            operands.append(partition_id_tensor())
        outs = _bass_exec_p.bind(
            *operands, out_avals=tuple(out_avals), in_names=tuple(all_names),
            out_names=tuple(out_names), lowering_input_output_aliases=(),
            sim_require_finite=True, sim_require_nnan=True, nc=nc)
        return tuple(outs)

    devices = jax.devices()[:N_CORES]
    mesh = Mesh(np.asarray(devices), ("core",))
    sharded = jax.jit(
        shard_map(_body, mesh=mesh,
                  in_specs=(PartitionSpec("core"),) * (n_params + n_outs),
                  out_specs=(PartitionSpec("core"),) * n_outs,
                  check_rep=False),
        keep_unused=True)
    sh = NamedSharding(mesh, PartitionSpec("core"))
    zeros_dev = [
        jax.device_put(np.zeros((N_CORES * z.shape[0], *z.shape[1:]), z.dtype), sh)
        for z in zero_outs]

    def run(in_maps):
        concat_in = [
            jax.device_put(
                np.concatenate([in_maps[c][n] for c in range(N_CORES)], axis=0), sh)
            for n in in_names]
        outs = sharded(*concat_in, *zeros_dev)
        res = []
        for c in range(N_CORES):
            m = {}
            for i, n in enumerate(out_names):
                rows = out_avals[i].shape[0]
                m[n] = np.asarray(outs[i][c * rows:(c + 1) * rows])
            res.append(m)
        return res

    _cache["runner"] = run
    return run


def kernel(enc_out, pred_out, W_enc, b_enc, W_pred, b_pred, W_joint, b_joint):
    nc = _get_nc()
    in_maps = make_in_maps(enc_out, pred_out, W_enc, b_enc, W_pred, b_pred,
                           W_joint, b_joint)
    if _axon_active():
        results = _get_fast_runner(nc)(in_maps)
    else:
        results = bass_utils.run_bass_kernel_spmd(
            nc, in_maps, list(range(N_CORES))).results
    return assemble(results)
